# revision 26
# baseline (speedup 1.0000x reference)
"""Trainium2 Bass kernel for nn_MultiHeadSelfAttention_29403346108551.

Reference semantics (faithful to the original nn.Module):
  q/k/v = (x @ W.T + b) .reshape(b, 16, 2048, 64)   # reshape, NOT transpose
  RoPE with a *scalar* position t=seq_len (same angle for every token),
  scores = q k^T / 8, softmax, o = p v, merge heads (real transpose), o @ wo.T + bo.

Key structural facts used for sharding:
  - The head split is a row-major reshape, so head h only touches rows
    [128h, 128h+128) of x, and its 2048 "time" steps are (r, c) -> t = r*16 + c
    with r = row-in-block, c = column-chunk (j // 64).
  - The RoPE rotation is a fixed per-column-pair linear map -> folded into
    wq / wk (and bq / bk) on the host.
  - Core cid handles batch cid//4 and head group cid%4 (4 heads = a contiguous
    512-row slice of x). The row-parallel output projection partials are summed
    on the host during the gather (4 cores per batch, 2 head-pair partials per
    core), along with bo.

On-device compute (per core), all matmuls in float32r (TF32-like, full PE rate):
  xT [1024, 512] (host-transposed) ->
  Q/K projections into per-head-pair transposed layout qT2/kT2 [128, 2048]
  (partitions = (head parity, d), columns = permuted time t'' = c*128 + r),
  V projection into v_aug [128, 16, 65] (65th column = ones -> softmax
  denominator accumulates for free in the PV matmul),
  S^T = k q^T per (head, chunk, window) -> exp on ACT (scale=1/8 folded in),
  o^T_aug = v_aug^T @ expS  (rows 0-63 = unnormalized o^T, row 64 = denom),
  normalize via a PSUM->SBUF copy + reshaped-DMA reciprocal + broadcast
  multiply (all off the PE critical path), final projection per head pair with
  o2T as the stationary operand, outputs written in t'' order (host un-permutes).
"""

import numpy as np

import concourse.bass as bass
import concourse.mybir as mybir
import concourse.tile as tile
from concourse import bacc
from concourse.bass_utils import run_bass_kernel_spmd

F32 = mybir.dt.float32
F32R = mybir.dt.float32r

MODEL_DIM = 1024
NUM_HEADS = 16
D_K = 64            # head dim
B = 2
T = 2048
N_CORES = 8
HPC = 4             # heads per core
RPC = 512           # x rows per core
NK = 8              # contraction chunks of 128 over MODEL_DIM
SEQ_POS = 2048      # scalar rope position used by the reference (== seq len)


def _round_fp32r(a: np.ndarray) -> np.ndarray:
    """Round fp32 to the fp32r grid (round-to-nearest-even, 12 low mantissa
    bits dropped) so the PE's fp32r read is exact."""
    b = a.astype(np.float32).view(np.uint32)
    r = (b + 0x7FF + ((b >> 12) & 1)) & np.uint32(0xFFFFF000)
    return r.view(np.float32)


def _bcast_p(ap, n):
    """Partition-step-0 broadcast AP (for DMA sources)."""
    return bass.AP(
        tensor=ap.tensor, offset=ap.offset,
        ap=[[0, n]] + [list(p) for p in ap.ap],
    )


def _build_program() -> bass.Bass:
    nc = bacc.Bacc(None, target_bir_lowering=False, debug=False)

    xT = nc.dram_tensor("xT", [MODEL_DIM, RPC], F32, kind="ExternalInput")
    wqT = nc.dram_tensor("wqT", [MODEL_DIM, MODEL_DIM], F32, kind="ExternalInput")
    wkT = nc.dram_tensor("wkT", [MODEL_DIM, MODEL_DIM], F32, kind="ExternalInput")
    wvT = nc.dram_tensor("wvT", [MODEL_DIM, MODEL_DIM], F32, kind="ExternalInput")
    woT = nc.dram_tensor("woT", [2, 128, MODEL_DIM], F32, kind="ExternalInput")
    bq = nc.dram_tensor("bq", [128, 8], F32, kind="ExternalInput")
    bk = nc.dram_tensor("bk", [128, 8], F32, kind="ExternalInput")
    bv = nc.dram_tensor("bv", [MODEL_DIM], F32, kind="ExternalInput")
    ones16 = nc.dram_tensor("ones16", [16], F32, kind="ExternalInput")
    # one partial per head pair; host sums (lets pair-0 final matmuls overlap
    # pair-1 attention instead of all serializing at the tail)
    outp = nc.dram_tensor("outp", [2, T, MODEL_DIM], F32, kind="ExternalOutput")

    with tile.TileContext(nc) as tc:
        with (
            tc.tile_pool(name="wpool", bufs=12) as wpool,
            tc.tile_pool(name="wopool", bufs=2) as wopool,
            tc.tile_pool(name="xpool", bufs=8) as xpool,
            tc.tile_pool(name="qkpool", bufs=2) as qkpool,
            tc.tile_pool(name="vpool", bufs=4) as vpool,
            tc.tile_pool(name="espool", bufs=4) as espool,
            tc.tile_pool(name="o2pool", bufs=2) as o2pool,
            tc.tile_pool(name="outpool", bufs=2) as outpool,
            tc.tile_pool(name="cpool", bufs=1) as cpool,
            tc.tile_pool(name="opool", bufs=2) as opool,
            tc.tile_pool(name="rcpool", bufs=2) as rcpool,
            tc.tile_pool(name="rcbig", bufs=1) as rcbig,
        ):
            # ---- constant / input loads ----
            xt = []
            for k in range(NK):
                t_ = xpool.tile([128, RPC], F32R, tag="xt", name=f"xt_{k}")
                nc.sync.dma_start(out=t_, in_=xT[k * 128:(k + 1) * 128, :].bitcast(F32R))
                xt.append(t_)

            wq_sb, wk_sb = [], []
            for k in range(NK):
                t_ = wpool.tile([128, MODEL_DIM], F32R, tag="w", name=f"wq_{k}")
                nc.scalar.dma_start(out=t_, in_=wqT[k * 128:(k + 1) * 128, :].bitcast(F32R))
                wq_sb.append(t_)
            for k in range(NK):
                t_ = wpool.tile([128, MODEL_DIM], F32R, tag="w", name=f"wk_{k}")
                nc.sync.dma_start(out=t_, in_=wkT[k * 128:(k + 1) * 128, :].bitcast(F32R))
                wk_sb.append(t_)

            bq_sb = cpool.tile([128, 8], F32)
            nc.sync.dma_start(out=bq_sb, in_=bq[:, :])
            bk_sb = cpool.tile([128, 8], F32)
            nc.sync.dma_start(out=bk_sb, in_=bk[:, :])
            bv_bc = cpool.tile([128, MODEL_DIM], F32)
            nc.sync.dma_start(out=bv_bc, in_=_bcast_p(bv[:], 128))
            ones_sb = cpool.tile([128, 16], F32R)
            nc.sync.dma_start(out=ones_sb, in_=_bcast_p(ones16[:], 128).bitcast(F32R))

            qT2a = qkpool.tile([128, 2 * T], F32R, tag="qk", name="qT2a")
            kT2a = qkpool.tile([128, 2 * T], F32R, tag="qk", name="kT2a")
            qT2 = [qT2a[:, 0:T], qT2a[:, T:2 * T]]
            kT2 = [kT2a[:, 0:T], kT2a[:, T:2 * T]]

            # ---- projections (own PSUM pool, closed before attention) ----
            # k-outer loops: each weight k-chunk is used by 8 back-to-back
            # matmuls then released, so weight DMA streams in parallel with
            # the compute instead of serializing on pool-slot release.
            with tc.tile_pool(name="psproj", bufs=8, space="PSUM") as psproj:
                wv_sb = []
                for k in range(NK):
                    t_ = wpool.tile([128, MODEL_DIM], F32R, tag="w", name=f"wv_{k}")
                    nc.scalar.dma_start(out=t_, in_=wvT[k * 128:(k + 1) * 128, :].bitcast(F32R))
                    wv_sb.append(t_)

                for w_sb, bias_sb, dsta in ((wq_sb, bq_sb, qT2a), (wk_sb, bk_sb, kT2a)):
                    psq = [psproj.tile([128, RPC], F32, tag="proj", name=f"psq_{p}")
                           for p in range(8)]
                    for k in range(NK):
                        for p in range(8):
                            nc.tensor.matmul(
                                psq[p], w_sb[k][:, p * 128:(p + 1) * 128], xt[k],
                                start=(k == 0), stop=(k == NK - 1),
                            )
                    for p in range(8):
                        # drain both m-halves in one 3D-AP op per (half, ph):
                        # dst[64ph+d, m*T + c*128 + r] <- ps[64half+d, (2m+ph)*128 + r]
                        for half in range(2):
                            c = 2 * p + half
                            for ph in range(2):
                                dst3 = dsta[64 * ph:64 * ph + 64, :].rearrange(
                                    "p (m t) -> p m t", m=2)[:, :, c * 128:(c + 1) * 128]
                                src3 = psq[p][64 * half:64 * half + 64, ph * 128:].rearrange(
                                    "p (b r) -> p b r", r=128)[:, 0:3:2, :]
                                nc.vector.tensor_scalar_add(
                                    dst3, src3,
                                    bias_sb[64 * half:64 * half + 64, p:p + 1],
                                )

                v_aug = []
                for bl in range(HPC):
                    va = vpool.tile([128, 16, 65], F32R, tag="va", name=f"v_aug_{bl}")
                    nc.vector.tensor_copy(
                        va[:, :, 64:65],
                        ones_sb.rearrange("p (a b) -> p a b", b=1),
                    )
                    v_aug.append(va)
                psv = [psproj.tile([128, RPC], F32, tag="proj", name=f"psv_{i}")
                       for i in range(8)]
                for k in range(NK):
                    for bl in range(HPC):
                        for jw in range(2):
                            nc.tensor.matmul(
                                psv[2 * bl + jw], xt[k][:, bl * 128:(bl + 1) * 128],
                                wv_sb[k][:, jw * 512:(jw + 1) * 512],
                                start=(k == 0), stop=(k == NK - 1),
                            )
                for bl in range(HPC):
                    for jw in range(2):
                        nc.vector.tensor_tensor(
                            v_aug[bl][:, 8 * jw:8 * jw + 8, 0:64],
                            psv[2 * bl + jw][:, :].rearrange("p (cc d) -> p cc d", d=64),
                            bv_bc[:, jw * 512:(jw + 1) * 512].rearrange(
                                "p (cc d) -> p cc d", d=64),
                            mybir.AluOpType.add,
                        )

            wo_sb = []
            for m_ in range(2):
                t_ = wopool.tile([128, MODEL_DIM], F32R, tag="wo", name=f"wo_{m_}")
                nc.scalar.dma_start(out=t_, in_=woT[m_, :, :].bitcast(F32R))
                wo_sb.append(t_)

            # ---- attention + per-pair final projection ----
            with (
                tc.tile_pool(name="psS", bufs=2, space="PSUM") as psS_pool,
                tc.tile_pool(name="psO", bufs=2, space="PSUM") as psO_pool,
            ):
                o2T = [o2pool.tile([128, T], F32R, tag="o2", name=f"o2T_{i}") for i in range(2)]

                def normalize(m, base, wq_, psO):
                    """psO [65, 512] -> o2T[m][base:base+64, wq_*512:+512]"""
                    QT_ = T // 4
                    o_sb = opool.tile([65, QT_], F32, tag="osb")
                    nc.vector.tensor_copy(o_sb, psO)
                    den_t = rcpool.tile([128, 4], F32, tag="dent")
                    nc.gpsimd.dma_start(
                        out=den_t,
                        in_=o_sb[64:65, :].rearrange("a (p i) -> a p i", p=128),
                    )
                    rcp_t = rcpool.tile([128, 4], F32, tag="rcpt")
                    nc.vector.reciprocal(rcp_t, den_t)
                    rcp_flat = rcbig.tile([1, QT_], F32, tag="rcpf")
                    nc.gpsimd.dma_start(
                        out=rcp_flat[0:1, :].rearrange("a (p i) -> a p i", p=128),
                        in_=rcp_t,
                    )
                    rcp_bc = rcbig.tile([64, QT_], F32, tag="rcpb")
                    nc.gpsimd.partition_broadcast(rcp_bc, rcp_flat)
                    nc.vector.tensor_tensor(
                        o2T[m][base:base + 64, wq_ * QT_:(wq_ + 1) * QT_],
                        o_sb[0:64, :], rcp_bc,
                        mybir.AluOpType.mult,
                    )

                # Software-pipelined attention over all (pair, tq-quarter, cc)
                # iterations: emit scores(i+1) BEFORE pv(i) so the in-order PE
                # always has independent matmuls while the exp round-trips.
                group_state = {}

                def emit_scores(m, wq_, cc):
                    if cc == 0:
                        group_state[(m, wq_)] = (
                            psO_pool.tile([65, 512], F32, tag="oA", name=f"psO_A_{m}_{wq_}"),
                            psO_pool.tile([65, 512], F32, tag="oB", name=f"psO_B_{m}_{wq_}"),
                        )
                    c0 = 2 * cc
                    psS_A = psS_pool.tile([128, 1024], F32, tag="s")
                    psS_B = psS_pool.tile([128, 1024], F32, tag="s")
                    for ci in range(2):
                        c = c0 + ci
                        # adjacent matmuls on row strips 0-63 / 64-127 overlap
                        nc.tensor.matmul(
                            psS_A[:, ci * 512:(ci + 1) * 512],
                            kT2[m][0:64, c * 128:(c + 1) * 128],
                            qT2[m][0:64, wq_ * 512:(wq_ + 1) * 512],
                            start=True, stop=True,
                        )
                        nc.tensor.matmul(
                            psS_B[:, ci * 512:(ci + 1) * 512],
                            kT2[m][64:128, c * 128:(c + 1) * 128],
                            qT2[m][64:128, wq_ * 512:(wq_ + 1) * 512],
                            start=True, stop=True,
                        )
                    eS_A = espool.tile([128, 1024], F32R, tag="es")
                    nc.scalar.activation(
                        eS_A, psS_A, mybir.ActivationFunctionType.Exp, scale=0.125)
                    eS_B = espool.tile([128, 1024], F32R, tag="es")
                    nc.scalar.activation(
                        eS_B, psS_B, mybir.ActivationFunctionType.Exp, scale=0.125)
                    return eS_A, eS_B

                def emit_pv(m, wq_, cc, eS_A, eS_B):
                    psO_A, psO_B = group_state[(m, wq_)]
                    hA, hB = 2 * m, 2 * m + 1
                    for ci in range(2):
                        c = 2 * cc + ci
                        nc.tensor.matmul(
                            psO_A, v_aug[hA][:, c, :], eS_A[:, ci * 512:(ci + 1) * 512],
                            start=(c == 0), stop=(c == 15),
                        )
                        nc.tensor.matmul(
                            psO_B, v_aug[hB][:, c, :], eS_B[:, ci * 512:(ci + 1) * 512],
                            start=(c == 0), stop=(c == 15),
                        )
                    if cc == 7:
                        normalize(m, 0, wq_, psO_A)
                        normalize(m, 64, wq_, psO_B)

                def final_half(m_, tt0):
                    for tt in (tt0, tt0 + 1):
                        out_sb = outpool.tile([128, MODEL_DIM], F32, tag="out")
                        ps = psS_pool.tile([128, 1024], F32, tag="s")
                        for jw in range(2):
                            nc.tensor.matmul(
                                ps[:, jw * 512:(jw + 1) * 512],
                                o2T[m_][:, tt * 128:(tt + 1) * 128],
                                wo_sb[m_][:, jw * 512:(jw + 1) * 512],
                                start=True, stop=True,
                            )
                        nc.vector.tensor_copy(out_sb, ps)
                        nc.sync.dma_start(
                            out=outp[m_, tt * 128:(tt + 1) * 128, :], in_=out_sb,
                        )

                iters = [(m, wq_, cc) for m in range(2) for wq_ in range(4)
                         for cc in range(8)]
                pending = None
                for i, it in enumerate(iters):
                    es = emit_scores(*it)
                    if pending is not None:
                        emit_pv(*pending[0], *pending[1])
                    pending = (it, es)
                    # final projection for the previous group's quarter,
                    # emitted a full group after its normalize so the chain
                    # latency is hidden; [128,1024] psF tiles halve the psS
                    # slot pressure
                    g_cur, cc_ = i // 8, i % 8
                    if g_cur >= 1 and cc_ in (4, 6):
                        gd = iters[8 * (g_cur - 1)]
                        final_half(gd[0], 4 * gd[1] + (0 if cc_ == 4 else 2))
                emit_pv(*pending[0], *pending[1])
                final_half(1, 12)
                final_half(1, 14)

    nc.compile()
    return nc


_NC_CACHE = None


def _get_program():
    global _NC_CACHE
    if _NC_CACHE is None:
        _NC_CACHE = _build_program()
    return _NC_CACHE


def _host_prep(inputs):
    x = np.asarray(inputs["x"], np.float32)
    wq = np.asarray(inputs["wq"], np.float32)
    wk = np.asarray(inputs["wk"], np.float32)
    wv = np.asarray(inputs["wv"], np.float32)
    wo = np.asarray(inputs["wo"], np.float32)
    bq = np.asarray(inputs["bq"], np.float32)
    bk = np.asarray(inputs["bk"], np.float32)
    bv = np.asarray(inputs["bv"], np.float32)
    rot_cos = np.asarray(inputs["rot_cos"], np.float32)
    rot_sin = np.asarray(inputs["rot_sin"], np.float32)

    cos = rot_cos[SEQ_POS]  # [32]
    sin = rot_sin[SEQ_POS]

    def rope_fold_w(w):
        wv_ = w.reshape(16, 32, 2, MODEL_DIM)
        ev = wv_[:, :, 0] * cos[None, :, None] - wv_[:, :, 1] * sin[None, :, None]
        od = wv_[:, :, 0] * sin[None, :, None] + wv_[:, :, 1] * cos[None, :, None]
        return np.stack([ev, od], axis=2).reshape(MODEL_DIM, MODEL_DIM)

    def rope_fold_b(b_):
        bv_ = b_.reshape(16, 32, 2)
        ev = bv_[:, :, 0] * cos[None, :] - bv_[:, :, 1] * sin[None, :]
        od = bv_[:, :, 0] * sin[None, :] + bv_[:, :, 1] * cos[None, :]
        return np.stack([ev, od], axis=2).reshape(MODEL_DIM)

    wq_r = rope_fold_w(wq)
    wk_r = rope_fold_w(wk)
    bq_r = rope_fold_b(bq)
    bk_r = rope_fold_b(bk)

    wqT = _round_fp32r(np.ascontiguousarray(wq_r.T))
    wkT = _round_fp32r(np.ascontiguousarray(wk_r.T))
    wvT = _round_fp32r(np.ascontiguousarray(wv.T))
    bq_sb = np.ascontiguousarray(bq_r.reshape(8, 128).T)
    bk_sb = np.ascontiguousarray(bk_r.reshape(8, 128).T)

    in_maps = []
    for cid in range(N_CORES):
        bi, g = cid // 4, cid % 4
        xTc = _round_fp32r(np.ascontiguousarray(x[bi, 512 * g:512 * (g + 1), :].T))
        woTc = np.stack(
            [
                np.ascontiguousarray(
                    wo[:, (4 * g + 2 * m) * 64:(4 * g + 2 * m + 2) * 64].T
                )
                for m in range(2)
            ]
        )
        in_maps.append({
            "xT": xTc,
            "wqT": wqT, "wkT": wkT, "wvT": wvT,
            "woT": _round_fp32r(woTc),
            "bq": bq_sb, "bk": bk_sb, "bv": bv,
            "ones16": np.ones(16, np.float32),
        })
    return in_maps, np.asarray(inputs["bo"], np.float32)


def _gather(results, bo):
    out = np.empty((B, T, MODEL_DIM), np.float32)
    for bi in range(B):
        acc = results[4 * bi]["outp"].astype(np.float32).sum(axis=0)
        for g in range(1, 4):
            acc += results[4 * bi + g]["outp"].sum(axis=0)
        # t'' = c*128 + r  ->  t = r*16 + c
        acc = acc.reshape(16, 128, MODEL_DIM).transpose(1, 0, 2).reshape(T, MODEL_DIM)
        out[bi] = acc + bo[None, :]
    return out


def _run(inputs, trace=False, **kw):
    nc = _get_program()
    in_maps, bo = _host_prep(inputs)
    res = run_bass_kernel_spmd(nc, in_maps, list(range(N_CORES)), trace=trace, **kw)
    return _gather(res.results, bo), res


def kernel(**inputs) -> np.ndarray:
    out, _ = _run(inputs)
    return out


# revision 27
# speedup vs baseline: 1.2096x; 1.2096x over previous
"""Trainium2 Bass kernel for nn_MultiHeadSelfAttention_29403346108551.

Reference semantics (faithful to the original nn.Module):
  q/k/v = (x @ W.T + b) .reshape(b, 16, 2048, 64)   # reshape, NOT transpose
  RoPE with a *scalar* position t=seq_len (same angle for every token),
  scores = q k^T / 8, softmax, o = p v, merge heads (real transpose), o @ wo.T + bo.

Key structural facts used for sharding:
  - The head split is a row-major reshape, so head h only touches rows
    [128h, 128h+128) of x, and its 2048 "time" steps are (r, c) -> t = r*16 + c
    with r = row-in-block, c = column-chunk (j // 64).
  - The RoPE rotation is a fixed per-column-pair linear map -> folded into
    wq / wk (and bq / bk) on the host.
  - Core cid handles batch cid//4 and head group cid%4 (4 heads = a contiguous
    512-row slice of x). The row-parallel output projection partials are summed
    on the host during the gather (4 cores per batch, 2 head-pair partials per
    core), along with bo.

On-device compute (per core), all matmuls in float32r (TF32-like, full PE rate):
  xT [1024, 512] (host-transposed) ->
  Q/K projections into per-head-pair transposed layout qT2/kT2 [128, 2048]
  (partitions = (head parity, d), columns = permuted time t'' = c*128 + r),
  V projection into v_aug [128, 16, 65] (65th column = ones -> softmax
  denominator accumulates for free in the PV matmul),
  S^T = k q^T per (head, chunk, window) -> exp on ACT (scale=1/8 folded in),
  o^T_aug = v_aug^T @ expS  (rows 0-63 = unnormalized o^T, row 64 = denom),
  normalize via a PSUM->SBUF copy + reshaped-DMA reciprocal + broadcast
  multiply (all off the PE critical path), final projection per head pair with
  o2T as the stationary operand, outputs written in t'' order (host un-permutes).
"""

import numpy as np

import concourse.bass as bass
import concourse.mybir as mybir
import concourse.tile as tile
from concourse import bacc
from concourse.bass_utils import run_bass_kernel_spmd

F32 = mybir.dt.float32
F32R = mybir.dt.float32r

MODEL_DIM = 1024
NUM_HEADS = 16
D_K = 64            # head dim
B = 2
T = 2048
N_CORES = 8
HPC = 4             # heads per core
RPC = 512           # x rows per core
NK = 8              # contraction chunks of 128 over MODEL_DIM
SEQ_POS = 2048      # scalar rope position used by the reference (== seq len)


def _round_fp32r(a: np.ndarray) -> np.ndarray:
    """Round fp32 to the fp32r grid (round-to-nearest-even, 12 low mantissa
    bits dropped) so the PE's fp32r read is exact."""
    b = a.astype(np.float32).view(np.uint32)
    r = (b + 0x7FF + ((b >> 12) & 1)) & np.uint32(0xFFFFF000)
    return r.view(np.float32)


def _bcast_p(ap, n):
    """Partition-step-0 broadcast AP (for DMA sources)."""
    return bass.AP(
        tensor=ap.tensor, offset=ap.offset,
        ap=[[0, n]] + [list(p) for p in ap.ap],
    )


def _build_program() -> bass.Bass:
    nc = bacc.Bacc(None, target_bir_lowering=False, debug=False)

    xT = nc.dram_tensor("xT", [MODEL_DIM, RPC], F32, kind="ExternalInput")
    wqT = nc.dram_tensor("wqT", [MODEL_DIM, MODEL_DIM], F32, kind="ExternalInput")
    wkT = nc.dram_tensor("wkT", [MODEL_DIM, MODEL_DIM], F32, kind="ExternalInput")
    wvT = nc.dram_tensor("wvT", [MODEL_DIM, MODEL_DIM], F32, kind="ExternalInput")
    woT = nc.dram_tensor("woT", [2, 128, MODEL_DIM], F32, kind="ExternalInput")
    bq = nc.dram_tensor("bq", [128, 8], F32, kind="ExternalInput")
    bk = nc.dram_tensor("bk", [128, 8], F32, kind="ExternalInput")
    bv = nc.dram_tensor("bv", [MODEL_DIM], F32, kind="ExternalInput")
    ones16 = nc.dram_tensor("ones16", [16], F32, kind="ExternalInput")
    # one partial per head pair; host sums (lets pair-0 final matmuls overlap
    # pair-1 attention instead of all serializing at the tail)
    outp = nc.dram_tensor("outp", [2, T, MODEL_DIM], F32, kind="ExternalOutput")

    with tile.TileContext(nc) as tc:
        with (
            tc.tile_pool(name="wpool", bufs=12) as wpool,
            tc.tile_pool(name="wopool", bufs=2) as wopool,
            tc.tile_pool(name="xpool", bufs=8) as xpool,
            tc.tile_pool(name="qkpool", bufs=2) as qkpool,
            tc.tile_pool(name="vpool", bufs=4) as vpool,
            tc.tile_pool(name="espool", bufs=4) as espool,
            tc.tile_pool(name="o2pool", bufs=2) as o2pool,
            tc.tile_pool(name="outpool", bufs=2) as outpool,
            tc.tile_pool(name="cpool", bufs=1) as cpool,
            tc.tile_pool(name="opool", bufs=2) as opool,
            tc.tile_pool(name="rcpool", bufs=2) as rcpool,
            tc.tile_pool(name="rcbig", bufs=1) as rcbig,
        ):
            # ---- constant / input loads ----
            xt = []
            for k in range(NK):
                t_ = xpool.tile([128, RPC], F32R, tag="xt", name=f"xt_{k}")
                nc.sync.dma_start(out=t_, in_=xT[k * 128:(k + 1) * 128, :].bitcast(F32R))
                xt.append(t_)

            wq_sb, wk_sb = [], []
            for k in range(NK):
                t_ = wpool.tile([128, MODEL_DIM], F32R, tag="w", name=f"wq_{k}")
                nc.scalar.dma_start(out=t_, in_=wqT[k * 128:(k + 1) * 128, :].bitcast(F32R))
                wq_sb.append(t_)
            for k in range(NK):
                t_ = wpool.tile([128, MODEL_DIM], F32R, tag="w", name=f"wk_{k}")
                nc.sync.dma_start(out=t_, in_=wkT[k * 128:(k + 1) * 128, :].bitcast(F32R))
                wk_sb.append(t_)

            bq_sb = cpool.tile([128, 8], F32)
            nc.sync.dma_start(out=bq_sb, in_=bq[:, :])
            bk_sb = cpool.tile([128, 8], F32)
            nc.sync.dma_start(out=bk_sb, in_=bk[:, :])
            bv_bc = cpool.tile([128, MODEL_DIM], F32)
            nc.sync.dma_start(out=bv_bc, in_=_bcast_p(bv[:], 128))
            ones_sb = cpool.tile([128, 16], F32R)
            nc.sync.dma_start(out=ones_sb, in_=_bcast_p(ones16[:], 128).bitcast(F32R))

            qT2a = qkpool.tile([128, 2 * T], F32R, tag="qk", name="qT2a")
            kT2a = qkpool.tile([128, 2 * T], F32R, tag="qk", name="kT2a")
            qT2 = [qT2a[:, 0:T], qT2a[:, T:2 * T]]
            kT2 = [kT2a[:, 0:T], kT2a[:, T:2 * T]]

            # ---- projections (own PSUM pool, closed before attention) ----
            # k-outer loops: each weight k-chunk is used by 8 back-to-back
            # matmuls then released, so weight DMA streams in parallel with
            # the compute instead of serializing on pool-slot release.
            with tc.tile_pool(name="psproj", bufs=8, space="PSUM") as psproj:
                wv_sb = []
                for k in range(NK):
                    t_ = wpool.tile([128, MODEL_DIM], F32R, tag="w", name=f"wv_{k}")
                    nc.scalar.dma_start(out=t_, in_=wvT[k * 128:(k + 1) * 128, :].bitcast(F32R))
                    wv_sb.append(t_)

                for w_sb, bias_sb, dsta in ((wq_sb, bq_sb, qT2a), (wk_sb, bk_sb, kT2a)):
                    psq = [psproj.tile([128, RPC], F32, tag="proj", name=f"psq_{p}")
                           for p in range(8)]
                    for k in range(NK):
                        for p in range(8):
                            nc.tensor.matmul(
                                psq[p], w_sb[k][:, p * 128:(p + 1) * 128], xt[k],
                                start=(k == 0), stop=(k == NK - 1),
                            )
                    for p in range(8):
                        # drain both m-halves in one 3D-AP op per (half, ph):
                        # dst[64ph+d, m*T + c*128 + r] <- ps[64half+d, (2m+ph)*128 + r]
                        for half in range(2):
                            c = 2 * p + half
                            for ph in range(2):
                                dst3 = dsta[64 * ph:64 * ph + 64, :].rearrange(
                                    "p (m t) -> p m t", m=2)[:, :, c * 128:(c + 1) * 128]
                                src3 = psq[p][64 * half:64 * half + 64, ph * 128:].rearrange(
                                    "p (b r) -> p b r", r=128)[:, 0:3:2, :]
                                nc.vector.tensor_scalar_add(
                                    dst3, src3,
                                    bias_sb[64 * half:64 * half + 64, p:p + 1],
                                )

                v_aug = []
                for bl in range(HPC):
                    va = vpool.tile([128, 16, 65], F32R, tag="va", name=f"v_aug_{bl}")
                    nc.vector.tensor_copy(
                        va[:, :, 64:65],
                        ones_sb.rearrange("p (a b) -> p a b", b=1),
                    )
                    v_aug.append(va)
                psv = [psproj.tile([128, RPC], F32, tag="proj", name=f"psv_{i}")
                       for i in range(8)]
                for k in range(NK):
                    for bl in range(HPC):
                        for jw in range(2):
                            nc.tensor.matmul(
                                psv[2 * bl + jw], xt[k][:, bl * 128:(bl + 1) * 128],
                                wv_sb[k][:, jw * 512:(jw + 1) * 512],
                                start=(k == 0), stop=(k == NK - 1),
                            )
                for bl in range(HPC):
                    for jw in range(2):
                        nc.vector.tensor_tensor(
                            v_aug[bl][:, 8 * jw:8 * jw + 8, 0:64],
                            psv[2 * bl + jw][:, :].rearrange("p (cc d) -> p cc d", d=64),
                            bv_bc[:, jw * 512:(jw + 1) * 512].rearrange(
                                "p (cc d) -> p cc d", d=64),
                            mybir.AluOpType.add,
                        )

            wo_sb = []
            for m_ in range(2):
                t_ = wopool.tile([128, MODEL_DIM], F32R, tag="wo", name=f"wo_{m_}")
                nc.scalar.dma_start(out=t_, in_=woT[m_, :, :].bitcast(F32R))
                wo_sb.append(t_)

            # ---- attention + per-pair final projection ----
            with (
                tc.tile_pool(name="psS", bufs=3, space="PSUM") as psS_pool,
                tc.tile_pool(name="psO", bufs=1, space="PSUM") as psO_pool,
            ):
                o2T = [o2pool.tile([128, T], F32R, tag="o2", name=f"o2T_{i}") for i in range(2)]

                def normalize(m, base, wq_, psO):
                    """psO [65, 512] -> o2T[m][base:base+64, wq_*512:+512]"""
                    QT_ = T // 4
                    o_sb = opool.tile([65, QT_], F32, tag="osb")
                    nc.vector.tensor_copy(o_sb, psO)
                    den_t = rcpool.tile([128, 4], F32, tag="dent")
                    nc.gpsimd.dma_start(
                        out=den_t,
                        in_=o_sb[64:65, :].rearrange("a (p i) -> a p i", p=128),
                    )
                    rcp_t = rcpool.tile([128, 4], F32, tag="rcpt")
                    nc.vector.reciprocal(rcp_t, den_t)
                    rcp_flat = rcbig.tile([1, QT_], F32, tag="rcpf")
                    nc.gpsimd.dma_start(
                        out=rcp_flat[0:1, :].rearrange("a (p i) -> a p i", p=128),
                        in_=rcp_t,
                    )
                    rcp_bc = rcbig.tile([64, QT_], F32, tag="rcpb")
                    nc.gpsimd.partition_broadcast(rcp_bc, rcp_flat)
                    nc.vector.tensor_tensor(
                        o2T[m][base:base + 64, wq_ * QT_:(wq_ + 1) * QT_],
                        o_sb[0:64, :], rcp_bc,
                        mybir.AluOpType.mult,
                    )

                # Software-pipelined attention over all (pair, tq-quarter, cc)
                # iterations: emit scores(i+1) BEFORE pv(i) so the in-order PE
                # always has independent matmuls while the exp round-trips.
                group_state = {}

                def emit_scores(m, wq_, cc):
                    if cc == 0:
                        group_state[(m, wq_)] = (
                            psO_pool.tile([65, 512], F32, tag="oA", name=f"psO_A_{m}_{wq_}"),
                            psO_pool.tile([65, 512], F32, tag="oB", name=f"psO_B_{m}_{wq_}"),
                        )
                    c0 = 2 * cc
                    psS_A = psS_pool.tile([128, 1024], F32, tag="s")
                    psS_B = psS_pool.tile([128, 1024], F32, tag="s")
                    for ci in range(2):
                        c = c0 + ci
                        # adjacent matmuls on row strips 0-63 / 64-127 overlap
                        nc.tensor.matmul(
                            psS_A[:, ci * 512:(ci + 1) * 512],
                            kT2[m][0:64, c * 128:(c + 1) * 128],
                            qT2[m][0:64, wq_ * 512:(wq_ + 1) * 512],
                            start=True, stop=True,
                        )
                        nc.tensor.matmul(
                            psS_B[:, ci * 512:(ci + 1) * 512],
                            kT2[m][64:128, c * 128:(c + 1) * 128],
                            qT2[m][64:128, wq_ * 512:(wq_ + 1) * 512],
                            start=True, stop=True,
                        )
                    eS_A = espool.tile([128, 1024], F32R, tag="es")
                    nc.scalar.activation(
                        eS_A, psS_A, mybir.ActivationFunctionType.Exp, scale=0.125)
                    eS_B = espool.tile([128, 1024], F32R, tag="es")
                    nc.scalar.activation(
                        eS_B, psS_B, mybir.ActivationFunctionType.Exp, scale=0.125)
                    return eS_A, eS_B

                def emit_pv(m, wq_, cc, eS_A, eS_B):
                    psO_A, psO_B = group_state[(m, wq_)]
                    hA, hB = 2 * m, 2 * m + 1
                    for ci in range(2):
                        c = 2 * cc + ci
                        nc.tensor.matmul(
                            psO_A, v_aug[hA][:, c, :], eS_A[:, ci * 512:(ci + 1) * 512],
                            start=(c == 0), stop=(c == 15),
                        )
                        nc.tensor.matmul(
                            psO_B, v_aug[hB][:, c, :], eS_B[:, ci * 512:(ci + 1) * 512],
                            start=(c == 0), stop=(c == 15),
                        )
                    if cc == 7:
                        normalize(m, 0, wq_, psO_A)
                        normalize(m, 64, wq_, psO_B)

                def final_half(m_, tt0):
                    for tt in (tt0, tt0 + 1):
                        out_sb = outpool.tile([128, MODEL_DIM], F32, tag="out")
                        ps = psS_pool.tile([128, 1024], F32, tag="s")
                        for jw in range(2):
                            nc.tensor.matmul(
                                ps[:, jw * 512:(jw + 1) * 512],
                                o2T[m_][:, tt * 128:(tt + 1) * 128],
                                wo_sb[m_][:, jw * 512:(jw + 1) * 512],
                                start=True, stop=True,
                            )
                        nc.vector.tensor_copy(out_sb, ps)
                        nc.sync.dma_start(
                            out=outp[m_, tt * 128:(tt + 1) * 128, :], in_=out_sb,
                        )

                iters = [(m, wq_, cc) for m in range(2) for wq_ in range(4)
                         for cc in range(8)]
                from collections import deque
                pend = deque()
                for i, it in enumerate(iters):
                    es = emit_scores(*it)
                    if len(pend) >= 2:
                        pp = pend.popleft()
                        emit_pv(*pp[0], *pp[1])
                    pend.append((it, es))
                    # final projection for the previous group's quarter,
                    # emitted a full group after its normalize so the chain
                    # latency is hidden; [128,1024] psF tiles halve the psS
                    # slot pressure
                    g_cur, cc_ = i // 8, i % 8
                    if g_cur >= 1 and cc_ in (4, 6):
                        gd = iters[8 * (g_cur - 1)]
                        final_half(gd[0], 4 * gd[1] + (0 if cc_ == 4 else 2))
                while pend:
                    pp = pend.popleft()
                    emit_pv(*pp[0], *pp[1])
                final_half(1, 12)
                final_half(1, 14)

    nc.compile()
    return nc


_NC_CACHE = None


def _get_program():
    global _NC_CACHE
    if _NC_CACHE is None:
        _NC_CACHE = _build_program()
    return _NC_CACHE


def _host_prep(inputs):
    x = np.asarray(inputs["x"], np.float32)
    wq = np.asarray(inputs["wq"], np.float32)
    wk = np.asarray(inputs["wk"], np.float32)
    wv = np.asarray(inputs["wv"], np.float32)
    wo = np.asarray(inputs["wo"], np.float32)
    bq = np.asarray(inputs["bq"], np.float32)
    bk = np.asarray(inputs["bk"], np.float32)
    bv = np.asarray(inputs["bv"], np.float32)
    rot_cos = np.asarray(inputs["rot_cos"], np.float32)
    rot_sin = np.asarray(inputs["rot_sin"], np.float32)

    cos = rot_cos[SEQ_POS]  # [32]
    sin = rot_sin[SEQ_POS]

    def rope_fold_w(w):
        wv_ = w.reshape(16, 32, 2, MODEL_DIM)
        ev = wv_[:, :, 0] * cos[None, :, None] - wv_[:, :, 1] * sin[None, :, None]
        od = wv_[:, :, 0] * sin[None, :, None] + wv_[:, :, 1] * cos[None, :, None]
        return np.stack([ev, od], axis=2).reshape(MODEL_DIM, MODEL_DIM)

    def rope_fold_b(b_):
        bv_ = b_.reshape(16, 32, 2)
        ev = bv_[:, :, 0] * cos[None, :] - bv_[:, :, 1] * sin[None, :]
        od = bv_[:, :, 0] * sin[None, :] + bv_[:, :, 1] * cos[None, :]
        return np.stack([ev, od], axis=2).reshape(MODEL_DIM)

    wq_r = rope_fold_w(wq)
    wk_r = rope_fold_w(wk)
    bq_r = rope_fold_b(bq)
    bk_r = rope_fold_b(bk)

    wqT = _round_fp32r(np.ascontiguousarray(wq_r.T))
    wkT = _round_fp32r(np.ascontiguousarray(wk_r.T))
    wvT = _round_fp32r(np.ascontiguousarray(wv.T))
    bq_sb = np.ascontiguousarray(bq_r.reshape(8, 128).T)
    bk_sb = np.ascontiguousarray(bk_r.reshape(8, 128).T)

    in_maps = []
    for cid in range(N_CORES):
        bi, g = cid // 4, cid % 4
        xTc = _round_fp32r(np.ascontiguousarray(x[bi, 512 * g:512 * (g + 1), :].T))
        woTc = np.stack(
            [
                np.ascontiguousarray(
                    wo[:, (4 * g + 2 * m) * 64:(4 * g + 2 * m + 2) * 64].T
                )
                for m in range(2)
            ]
        )
        in_maps.append({
            "xT": xTc,
            "wqT": wqT, "wkT": wkT, "wvT": wvT,
            "woT": _round_fp32r(woTc),
            "bq": bq_sb, "bk": bk_sb, "bv": bv,
            "ones16": np.ones(16, np.float32),
        })
    return in_maps, np.asarray(inputs["bo"], np.float32)


def _gather(results, bo):
    out = np.empty((B, T, MODEL_DIM), np.float32)
    for bi in range(B):
        acc = results[4 * bi]["outp"].astype(np.float32).sum(axis=0)
        for g in range(1, 4):
            acc += results[4 * bi + g]["outp"].sum(axis=0)
        # t'' = c*128 + r  ->  t = r*16 + c
        acc = acc.reshape(16, 128, MODEL_DIM).transpose(1, 0, 2).reshape(T, MODEL_DIM)
        out[bi] = acc + bo[None, :]
    return out


def _run(inputs, trace=False, **kw):
    nc = _get_program()
    in_maps, bo = _host_prep(inputs)
    res = run_bass_kernel_spmd(nc, in_maps, list(range(N_CORES)), trace=trace, **kw)
    return _gather(res.results, bo), res


def kernel(**inputs) -> np.ndarray:
    out, _ = _run(inputs)
    return out
